# revision 44
# baseline (speedup 1.0000x reference)
"""Trainium2 Bass kernel for nn_Encoder_Block (dense transformer encoder block).

Strategy: pure data parallel across 8 NeuronCores (B=16 -> 2 batch elems per
core), all weights replicated.  Entire block computed on-chip per batch elem:
  x + pos -> LN0 (res) -> 4x [dsconv -> relu -> +res -> LN] -> attention
  -> +res -> LNe -> FC -> relu -> +res

v2 redesign (vs the f32r baseline):
  - bf16 matmul operands everywhere (1 cycle/row on PE, same as f32r, but
    halves weight DMA and unlocks the DVE 4x perf mode for the layernorm
    normalize writes).  Numerics validated host-side: rel err ~4e-3 << 2e-2.
  - q|k projected by one [C,128] stationary (q on psum partitions 0:64, k on
    64:128); score matmuls use the k chunks as stationary from the
    partition-offset region.
  - Attention for the two batch elems is chunk-interleaved (j-loop covers
    both), so the ACT-bound softmax exp of one batch overlaps the other's
    score/PV matmuls.
  - LN statistics cross-partition reduce+broadcast via a single ones[C,C]
    matmul; rsqrt stays exp(-0.5*ln(var+eps)) in the pinned act table set.
  - Attention denominator reciprocal on DVE (vector.reciprocal) instead of
    ACT ln/exp.
  - PE p-state: dummy warm-up matmuls run during the prologue DMA so the
    first conv already executes at full clock; conv emission is software-
    pipelined (layer l+1 of batch 0 is emitted before the LN tail of layer
    l batch 1) so the PE queue never stalls on LN statistics.
"""

import sys

sys.path.insert(0, "/opt/trn_rl_repo")

import math

import numpy as np

import concourse.bass as bass
import concourse.tile as tile
from concourse import bacc, bass_isa, mybir
from concourse.bass_utils import run_bass_kernel_spmd

F32 = mybir.dt.float32
F32R = mybir.dt.float32r
BF16 = mybir.dt.bfloat16
AF = mybir.ActivationFunctionType
ALU = mybir.AluOpType

B, C, T = 16, 128, 1024
NCONV, KW = 4, 7
DK = C // 2
NCORES = 8
BPC = B // NCORES          # batch elems per core
EPS = 1e-5
NEL = float(C * T)         # layernorm element count
PADT = T + KW - 1          # 1030: per-batch padded row in xpad
NLN = NCONV + 2            # LN0, 4 conv LNs, LNe
H = 512                    # half of T


def _pos_encoding() -> np.ndarray:
    i = np.arange(C)
    exp = -((i - (i % 2)).astype(np.float32) / np.float32(C))
    freqs = (np.float32(10000.0) ** exp)[:, None].astype(np.float32)
    phases = ((i % 2).astype(np.float32) * np.float32(np.pi / 2))[:, None]
    pos = np.arange(T, dtype=np.float32)[None, :]
    return np.sin(pos * freqs + phases).astype(np.float32)


def _uniform_val(a: np.ndarray):
    """Return the scalar if all elements equal, else None."""
    v = a.flat[0]
    return float(v) if np.all(a == v) else None


class _Flags:
    """Kernel-structure flags derived from host inspection of the inputs."""

    def __init__(self, ln_gc, ln_bc, b2_zero, fcb_zero, mask_ones):
        self.ln_gc = tuple(ln_gc)
        self.ln_bc = tuple(ln_bc)
        self.b2_zero = b2_zero
        self.fcb_zero = fcb_zero
        self.mask_ones = mask_ones

    def key(self):
        return (self.ln_gc, self.ln_bc, self.b2_zero, self.fcb_zero,
                self.mask_ones)


class _Bacc(bacc.Bacc):
    """Bacc with activation-table choice pinned to the one set that covers
    every function this kernel uses (ln/exp/square/copy/relu/identity), so
    exactly one table load is emitted."""

    _OURS = {AF.Ln, AF.Exp, AF.Square, AF.Copy, AF.Identity, AF.Relu}
    _KEEP = "natural_log_exp_and_others"

    def insert_act_table_loads(self):
        from concourse.bacc import _bass_rust, get_activation_tables
        has_activation = any(
            isinstance(i, mybir.InstActivation)
            for b in self.main_func.blocks
            for i in b.instructions
        )
        if not has_activation:
            return
        tables = [
            (nm, fs if nm == self._KEEP else (fs - self._OURS))
            for nm, fs in get_activation_tables(self.m.arch).items()
        ]
        _bass_rust.insert_act_table_loads(self, tables)


N_WARMUP = 46   # dummy PE matmuls covering the prologue DMA window


def _build_fast(flags: _Flags):
    """Fast path for uniform layernorm params + all-ones mask.

    Layernorms are never materialized: every LN output is an affine map
    s*z + t of the pre-LN tensor z (s, t runtime scalars from the stats),
    folded into the consumer:
      - conv_l(LN(z)) = relu(s*psum + t*rowsum(W2_l) [+b2]) via ACT
        scale/bias operands; xpad holds the raw z stream with pad columns
        set to mu (so LN(pad)=0 reproduces 'same' zero padding),
      - q/k projections: psum evicted with (s, t*colsum(W)) scale/bias,
      - v: scaled by s at eviction; its bias term is deferred through
        attention (a constant vector) and folded into the residual add as
        t*(Wo_eff^T colsum(Wv)),
      - FC: relu(s*psf + t*colsum(fc_w) [+fcb]) via ACT.
    LN0 (residual-only) contributes s0*zx via the conv0 residual add; its
    t0 is a constant shift of the whole residual stream, invariant under
    every later op, and is added back in the final output op.
    """
    nc = _Bacc("TRN2", target_bir_lowering=False, debug=False,
               num_devices=NCORES)

    def dram(name, shape, kind="ExternalInput", dtype=F32):
        return nc.dram_tensor(name, shape, dtype, kind=kind).ap()

    x_d = dram("x", [BPC, C, T])
    pos_d = dram("pos", [C, T], dtype=BF16)
    w2_d = dram("w2", [C, NCONV * KW * C], dtype=BF16)
    wqkv_d = dram("wqkv", [C, 3 * DK], dtype=BF16)
    wo_d = dram("wo", [DK, C], dtype=BF16)
    fcw_d = dram("fcw", [C, C], dtype=BF16)
    # folded column/row sums: [w2rsum_l1..l3 (+b2), wqsum, wksum,
    #  wowvsum, fcsum (+fcb)]
    sums_d = dram("sums", [C, 7])
    out_d = dram("out", [BPC, C, T], kind="ExternalOutput", dtype=BF16)

    from contextlib import ExitStack

    with tile.TileContext(nc) as tc, ExitStack() as ctx:
        cst = ctx.enter_context(tc.tile_pool(name="cst", bufs=1))
        big = ctx.enter_context(tc.tile_pool(name="bigbuf", bufs=1))
        work = ctx.enter_context(tc.tile_pool(name="work", bufs=2))
        tiny = ctx.enter_context(tc.tile_pool(name="tiny", bufs=4))
        ps_big = ctx.enter_context(
            tc.tile_pool(name="ps_big", bufs=2, space="PSUM"))
        ps_sm = ctx.enter_context(
            tc.tile_pool(name="ps_sm", bufs=2, space="PSUM"))

        _ctr = [0]

        def sm_tile(shape):
            _ctr[0] += 1
            return ps_sm.tile(shape, F32, tag="ps_sm", name=f"sm{_ctr[0]}")

        def ttile(shape, tag, dtype=F32, pool=None, bufs=None):
            _ctr[0] += 1
            kw = {} if bufs is None else {"bufs": bufs}
            return (pool or tiny).tile(shape, dtype, tag=tag,
                                       name=f"t{_ctr[0]}", **kw)

        # ---- tiny constants ----
        ones128 = cst.tile([C, C], BF16, tag="ones128")
        nc.vector.memset(ones128[:], 1.0)
        ones_row_b = cst.tile([1, C], BF16, tag="ones_row_b")
        nc.vector.memset(ones_row_b[:], 1.0)
        ones3 = cst.tile([C, 3], BF16, tag="ones3")
        nc.vector.memset(ones3[:], 1.0)
        eps_t = cst.tile([C, 1], F32, tag="eps_t")
        nc.vector.memset(eps_t[:], EPS)

        # ---- PE warm-up during the prologue DMA ----
        for _ in range(N_WARMUP):
            wps = sm_tile([1, C])
            nc.tensor.matmul(wps[:], ones128[:, 0:1], ones128[:],
                             start=True, stop=True)

        # ---- DMA (transfers serialize; order by criticality) ----
        pos_sb = cst.tile([C, T], BF16, tag="pos")
        nc.sync.dma_start(pos_sb[:], pos_d[:])
        xin = big.tile([C, BPC * T], F32, tag="xin")
        XSPL = 520            # matches the x+pos chunk boundary
        nc.sync.dma_start(xin[:, 0:XSPL], x_d[0][:, 0:XSPL])
        w2_sb = cst.tile([C, NCONV * KW * C], BF16, tag="w2")
        LW = KW * C
        nc.sync.dma_start(w2_sb[:, 0:LW], w2_d[:, 0:LW])
        nc.sync.dma_start(xin[:, XSPL:T], x_d[0][:, XSPL:T])
        for h in range(2):
            nc.sync.dma_start(xin[:, T + h * H:T + (h + 1) * H],
                              x_d[1][:, h * H:(h + 1) * H])
        for li in range(1, NCONV):
            nc.sync.dma_start(w2_sb[:, li * LW:(li + 1) * LW],
                              w2_d[:, li * LW:(li + 1) * LW])
        sums_sb = cst.tile([C, 7], F32, tag="sums")
        nc.sync.dma_start(sums_sb[:], sums_d[:])
        wqkv_sb = cst.tile([C, 3 * DK], BF16, tag="wqkv")
        nc.sync.dma_start(wqkv_sb[:], wqkv_d)
        wo_sb = cst.tile([DK, C], BF16, tag="wo")
        nc.sync.dma_start(wo_sb[:], wo_d)
        fcw_sb = cst.tile([C, C], BF16, tag="fcw")
        nc.sync.dma_start(fcw_sb[:], fcw_d)

        # xpad: ping-pong padded z-stream buffers per batch elem.
        # parity p slot of batch b: [0^3|z|0^3] at (p*BPC+b)*PADT.
        xpad = big.tile([C, 2 * BPC * PADT], BF16, tag="xpad")
        for s in range(2 * BPC):
            nc.gpsimd.memset(xpad[:, s * PADT:s * PADT + 3], 0.0)
            nc.gpsimd.memset(xpad[:, s * PADT + 3 + T:(s + 1) * PADT], 0.0)

        sq_scr = big.tile([C, T], BF16, tag="sq_scr")
        sq2_scr = big.tile([C, T], BF16, tag="sq2_scr")
        out_sb = big.tile([C, BPC * T], BF16, tag="out_sb")

        def zslot(par, b, off=0, n=T):
            base = (par * BPC + b) * PADT + 3
            return xpad[:, base + off: base + off + n]

        def pad_aps(par, b):
            base = (par * BPC + b) * PADT
            return (xpad[:, base:base + 3],
                    xpad[:, base + 3 + T:(par * BPC + b + 1) * PADT])

        # per-(ln_idx, b) runtime scalars
        S = {}      # s = g*rs  [C,1]
        TS_ = {}    # t = -s*mu (+bc)  [C,1]
        MU = {}     # mu [C,1]

        def emit_stats_sq(z_ap, stats_b, col=1, off=0):
            """S2 of z via DVE square + accumulating identity."""
            n = z_ap.shape[-1]
            nc.vector.tensor_tensor(sq_scr[:, off:off + n], z_ap, z_ap,
                                    ALU.mult)
            nc.vector.tensor_scalar(sq2_scr[:, off:off + n],
                                    sq_scr[:, off:off + n], 1.0, 0.0,
                                    op0=ALU.mult, op1=ALU.add,
                                    accum_out=stats_b[:, col:col + 1])

        def emit_ln_tail(ln_idx, b, stats_b):
            """stats -> (s, t, mu) for (ln_idx, b).  All-tiny ops.  The
            (s, t, mu) tiles get per-ln_idx tags (bufs=2 for the two batch
            elems) since some are read much later (t0 at the output op)."""
            S4 = ttile([C, 4], "S4")
            nc.gpsimd.partition_all_reduce(S4[:], stats_b[:], channels=C,
                                           reduce_op=bass_isa.ReduceOp.add)
            Ssum = ttile([C, 2], "Ssum")
            nc.vector.tensor_tensor(Ssum[:], S4[:, 0:2], S4[:, 2:4],
                                    ALU.add)
            mu = ttile([C, 1], f"mu{ln_idx}", bufs=2)
            nc.vector.tensor_scalar_mul(mu[:], Ssum[:, 0:1], 1.0 / NEL)
            # nvar = S1*mu - S2 = -N*var
            nvar = ttile([C, 1], "nvar")
            nc.vector.scalar_tensor_tensor(nvar[:], Ssum[:, 0:1], mu[:],
                                           Ssum[:, 1:2],
                                           op0=ALU.mult, op1=ALU.subtract)
            lnv = ttile([C, 1], "lnv")
            nc.scalar.activation(lnv[:], nvar[:], AF.Ln, scale=-1.0 / NEL,
                                 bias=eps_t[:])
            gc, bc = flags.ln_gc[ln_idx], flags.ln_bc[ln_idx]
            rs = ttile([C, 1], f"rs{ln_idx}", bufs=2)
            expb = math.log(gc) if gc > 0.0 else 0.0
            nc.scalar.activation(rs[:], lnv[:], AF.Exp, scale=-0.5,
                                 bias=float(expb))
            if gc <= 0.0 and gc != 1.0:
                rs2 = ttile([C, 1], f"rs2{ln_idx}", bufs=2)
                nc.vector.tensor_scalar_mul(rs2[:], rs[:], gc)
                rs = rs2
            # t = -(s*mu) (+bc)
            tt = ttile([C, 1], f"tt{ln_idx}", bufs=2)
            nc.vector.tensor_scalar(tt[:], mu[:], rs[:], -1.0,
                                    op0=ALU.mult, op1=ALU.mult)
            if bc != 0.0:
                tt2 = ttile([C, 1], f"tt2{ln_idx}", bufs=2)
                nc.vector.tensor_scalar_add(tt2[:], tt[:], bc)
                tt = tt2
            S[(ln_idx, b)] = rs
            TS_[(ln_idx, b)] = tt
            MU[(ln_idx, b)] = mu

        def emit_pads(par, b, mu):
            """Write mu into the 6 pad cols of slot (par, b).  On gpsimd:
            off the critical path (edge taps run last) and off DVE."""
            left, right = pad_aps(par, b)
            nc.gpsimd.tensor_scalar(left, ones3[:], mu[:], None,
                                    op0=ALU.mult)
            nc.gpsimd.tensor_scalar(right, ones3[:], mu[:], None,
                                    op0=ALU.mult)

        def conv_bias(li, b):
            """bias = t_{li} * rowsum(W2_li) (+ b2_li) for conv li>=1."""
            bias = ttile([C, 1], "cbias")
            nc.vector.tensor_scalar(bias[:], sums_sb[:, li - 1:li],
                                    TS_[(li, b)][:], None, op0=ALU.mult)
            return bias

        # ---- x + pos (zx into parity-0 slots) + LN0 stats.  Chunked at
        # col 520 so conv0's first-half matmuls (reading cols < 518) start
        # before the tail chunk is written. ----
        stats_x = []
        for b in range(BPC):
            stats_b = ttile([C, 4], "stats")
            for ci, (c0, c1) in enumerate(((0, 520), (520, T))):
                nc.vector.scalar_tensor_tensor(
                    zslot(0, b, c0, c1 - c0),
                    xin[:, b * T + c0:b * T + c1], 1.0, pos_sb[:, c0:c1],
                    op0=ALU.mult, op1=ALU.add,
                    accum_out=stats_b[:, 2 * ci:2 * ci + 1])
                emit_stats_sq(zslot(0, b, c0, c1 - c0), stats_b,
                              2 * ci + 1, c0)
            stats_x.append(stats_b)

        # ---- conv layers ----
        def emit_conv_mm(li, b):
            par = li % 2
            if b == 0:
                psc = ps_big.tile([C, T], F32, tag="ps_main",
                                  name=f"psc{li}_{b}")
            else:
                psc = ps_sm.tile([C, T], F32, tag="ps_sm",
                                 name=f"psc{li}_{b}")
            base = (par * BPC + b) * PADT
            # edge taps last so the pad writes (which need mu of this
            # layer's input) stay off the critical path
            for h in range(2):
                order = ([3, 4, 5, 6, 0, 1, 2] if h == 0
                         else [0, 1, 2, 3, 4, 5, 6])
                for i, d in enumerate(order):
                    nc.tensor.matmul(
                        psc[:, h * H:(h + 1) * H],
                        w2_sb[:, (li * KW + d) * C:(li * KW + d + 1) * C],
                        xpad[:, base + h * H + d: base + h * H + d + H],
                        start=(i == 0), stop=(i == KW - 1))
            return psc

        def emit_conv_evict(li, b, psc):
            """psum -> relu(LN-fold) -> z_{li+1} (other parity slot) with
            stats accumulation; per 512-half so the first half's eviction
            overlaps the second half's matmuls."""
            stats_b = ttile([C, 4], "stats")
            relu_t = ttile([C, T], f"relu", BF16, pool=work)
            bias = None if li == 0 else conv_bias(li, b)
            for h in range(2):
                sl = slice(h * H, (h + 1) * H)
                if li == 0:
                    nc.scalar.activation(relu_t[:, sl], psc[:, sl],
                                         AF.Relu)
                    nc.vector.scalar_tensor_tensor(
                        zslot(1, b, h * H, H), zslot(0, b, h * H, H),
                        S[(0, b)][:], relu_t[:, sl],
                        op0=ALU.mult, op1=ALU.add,
                        accum_out=stats_b[:, 2 * h:2 * h + 1])
                else:
                    nc.scalar.activation(relu_t[:, sl], psc[:, sl],
                                         AF.Relu, scale=S[(li, b)][:],
                                         bias=bias[:])
                    nc.vector.scalar_tensor_tensor(
                        zslot((li + 1) % 2, b, h * H, H), relu_t[:, sl],
                        1.0, zslot(li % 2, b, h * H, H),
                        op0=ALU.mult, op1=ALU.add,
                        accum_out=stats_b[:, 2 * h:2 * h + 1])
            for h in range(2):
                sl = slice(h * H, (h + 1) * H)
                if b == 0 or li == NCONV - 1:
                    nc.scalar.activation(
                        sq_scr[:, sl], zslot((li + 1) % 2, b, h * H, H),
                        AF.Square, accum_out=stats_b[:, 2 * h + 1:2 * h + 2])
                else:
                    emit_stats_sq(zslot((li + 1) % 2, b, h * H, H),
                                  stats_b, 2 * h + 1, h * H)
            return stats_b

        def conv_tail(li, b, stats_b):
            # stats of z_{li+1} -> (s,t,mu) for ln_idx li+1; pads for the
            # slot holding z_{li+1}
            emit_ln_tail(li + 1, b, stats_b)
            emit_pads((li + 1) % 2, b, MU[(li + 1, b)])

        pending = None
        for li in range(NCONV):
            psc0 = emit_conv_mm(li, 0)
            if li == 0:
                for b in range(BPC):
                    emit_ln_tail(0, b, stats_x[b])
            elif pending is not None:
                conv_tail(*pending)
                pending = None
            st0 = emit_conv_evict(li, 0, psc0)
            psc1 = emit_conv_mm(li, 1)
            conv_tail(li, 0, st0)
            st1 = emit_conv_evict(li, 1, psc1)
            pending = (li, 1, st1)

        # ---- attention (z4 in parity-0 slots; LN4 folded into q/k/v) ----
        ZP = NCONV % 2          # parity of z4 slots (= 0)
        qk = [None] * BPC
        vt = [None] * BPC
        eT = [None] * BPC
        psa = [None] * BPC
        stats_a = [None] * BPC

        def emit_qkv(b):
            xa = zslot(ZP, b)
            s4, t4 = S[(4, b)], TS_[(4, b)]
            bq = ttile([DK, 1], "bq")
            nc.vector.tensor_scalar(bq[:], sums_sb[0:DK, 3:4],
                                    t4[0:DK, :], None, op0=ALU.mult)
            bk = ttile([DK, 1], "bk")
            nc.vector.tensor_scalar(bk[:], sums_sb[0:DK, 4:5],
                                    t4[0:DK, :], None, op0=ALU.mult)
            qk_b = ttile([DK, 2 * T], "qk", BF16, pool=work)
            psq = ps_big.tile([DK, T], F32, tag="ps_main", name=f"psq{b}")
            psk = ps_big.tile([DK, T], F32, tag="ps_main", name=f"psk{b}")
            # per-half: each half of q/k only needs that half of z4, so the
            # first projections/evictions overlap the second-half residual
            for h in range(2):
                sl = slice(h * H, (h + 1) * H)
                nc.tensor.matmul(psq[:, sl], wqkv_sb[:, 0:DK],
                                 xa[:, h * H:h * H + H],
                                 start=True, stop=True)
                nc.tensor.matmul(psk[:, sl], wqkv_sb[:, DK:2 * DK],
                                 xa[:, h * H:h * H + H],
                                 start=True, stop=True)
                nc.vector.tensor_scalar(qk_b[:, sl], psq[:, sl],
                                        s4[0:DK, :], bq[:],
                                        op0=ALU.mult, op1=ALU.add)
                nc.scalar.activation(qk_b[:, T + h * H:T + (h + 1) * H],
                                     psk[:, sl], AF.Identity,
                                     scale=s4[0:DK, :], bias=bk[:])
            qk[b] = qk_b
            psv = sm_tile([C, H])
            for j in range(8):
                nc.tensor.matmul(psv[:, j * DK:(j + 1) * DK],
                                 xa[:, j * C:(j + 1) * C],
                                 wqkv_sb[:, 2 * DK:3 * DK],
                                 start=True, stop=True)
            vt_b = ttile([C, 8, DK + 1], "vt", BF16, pool=work)
            nc.gpsimd.memset(vt_b[:, :, DK:DK + 1], 1.0)
            # v scaled by s4; its bias is deferred past attention
            nc.vector.tensor_scalar(
                vt_b[:, :, 0:DK],
                psv[:].rearrange("p (j k) -> p j k", k=DK),
                s4[:], None, op0=ALU.mult)
            vt[b] = vt_b
            eT[b] = ttile([C, 8 * T], "eT", BF16, pool=work)
            stats_a[b] = ttile([C, 4], "stats")

        emit_qkv(0)
        if pending is not None:
            conv_tail(*pending)
            pending = None

        # scores/exp/PV: batch 1 lags batch 0 by LAG chunks so batch 0's
        # (latency-bound) tail chain overlaps batch 1's remaining softmax.
        LAG = 5

        def emit_att_step(j, b):
            pss = ps_big.tile([C, T], F32, tag="ps_main",
                              name=f"pss{j}_{b}")
            for h in range(2):
                nc.tensor.matmul(pss[:, h * H:(h + 1) * H],
                                 qk[b][:, T + j * C:T + (j + 1) * C],
                                 qk[b][:, h * H:h * H + H],
                                 start=True, stop=True)
            nc.scalar.activation(eT[b][:, j * T:(j + 1) * T], pss[:],
                                 AF.Exp)
            if j == 0:
                psa[b] = ps_sm.tile([C, T], F32, tag="ps_sm",
                                    name=f"psa{b}")
            for h in range(2):
                nc.tensor.matmul(
                    psa[b][0:DK + 1, h * H:(h + 1) * H], vt[b][:, j, :],
                    eT[b][:, j * T + h * H: j * T + h * H + H],
                    start=(j == 0), stop=(j == 7))

        _steps = []
        for step in range(8 + LAG):
            if step < 8:
                _steps.append((step, 0))
            if step >= LAG:
                _steps.append((step - LAG, 1))

        def emit_attn_tail(b):
            """Denominator reciprocal, normalize, Wo and residual for one
            batch.  The psr broadcast and pso (Wo output) REUSE psa[b]'s own
            two banks (the av/denominator data there is dead once read), so
            the tail allocates no psum and never blocks the score/PV
            pipeline of the other batch."""
            P = psa[b]
            av = ttile([DK, T], "av", BF16, pool=work)
            rr = ttile([1, T], "rr", BF16, pool=tiny, bufs=2)
            avn = ttile([DK, T], "avn", BF16, pool=work)
            t4 = TS_[(4, b)]
            ba = ttile([C, 1], "ba")
            nc.vector.scalar_tensor_tensor(ba[:], sums_sb[:, 5:6], t4[:],
                                           TS_[(0, b)][:],
                                           op0=ALU.mult, op1=ALU.add)
            # reads of psa (per half); bf16 reciprocal of the softmax
            # denominator is a pure normalization factor (0.4% rounding,
            # well inside tolerance).  batch 1 runs after the softmax so its
            # av copies go to the then-idle ACT engine.
            # psr/pso borrow dead psum banks: batch 0 reuses its own psa
            # (all its reads must be emitted before the writes), batch 1
            # borrows batch 0's fully-dead psa so each half chains
            # immediately after its own reads.
            R = psa[0]
            psr0 = R[0:DK, 0:H]          # bank0 reuse
            pso0 = R[0:C, H:T]           # bank1 reuse
            if b == 0:
                for h in range(2):
                    sl = slice(h * H, (h + 1) * H)
                    with nc.allow_low_precision("bf16 denominator"):
                        nc.vector.reciprocal(rr[:, sl], P[DK:DK + 1, sl])
                    nc.vector.tensor_copy(av[:, sl], P[0:DK, sl])
            for h in range(2):
                sl = slice(h * H, (h + 1) * H)
                if b == 1:
                    with nc.allow_low_precision("bf16 denominator"):
                        nc.vector.reciprocal(rr[:, sl], P[DK:DK + 1, sl])
                    nc.scalar.copy(av[:, sl], P[0:DK, sl])
                nc.tensor.matmul(psr0, ones_row_b[:, 0:DK],
                                 rr[:, sl], start=True, stop=True)
                nc.vector.tensor_tensor(avn[:, sl], av[:, sl],
                                        psr0, ALU.mult)
                nc.tensor.matmul(pso0, wo_sb[:], avn[:, sl],
                                 start=True, stop=True)
                nc.vector.scalar_tensor_tensor(
                    zslot(1, b, h * H, H), zslot(ZP, b, h * H, H), ba[:],
                    pso0, op0=ALU.add, op1=ALU.add,
                    accum_out=stats_a[b][:, 2 * h:2 * h + 1])
            for h in range(2):
                sl = slice(h * H, (h + 1) * H)
                if b == 0:
                    emit_stats_sq(zslot(1, b, h * H, H), stats_a[b],
                                  2 * h + 1, h * H)
                else:
                    nc.scalar.activation(
                        sq_scr[:, sl], zslot(1, b, h * H, H), AF.Square,
                        accum_out=stats_a[b][:, 2 * h + 1:2 * h + 2])

        def emit_fc_mm(b):
            psf = ps_big.tile([C, T], F32, tag="ps_main", name=f"psf{b}")
            for h in range(2):
                nc.tensor.matmul(psf[:, h * H:(h + 1) * H], fcw_sb[:],
                                 zslot(1, b, h * H, H), start=True,
                                 stop=True)
            return psf

        def emit_fc_evict(b, psf):
            s5, t5 = S[(5, b)], TS_[(5, b)]
            fbias = ttile([C, 1], "fbias")
            nc.vector.tensor_scalar(fbias[:], sums_sb[:, 6:7], t5[:], None,
                                    op0=ALU.mult)
            relu_f = ttile([C, T], "relu_fc", BF16, pool=work)
            for h in range(2):
                sl = slice(h * H, (h + 1) * H)
                nc.scalar.activation(relu_f[:, sl], psf[:, sl], AF.Relu,
                                     scale=s5[:], bias=fbias[:])
                ob = out_sb[:, b * T:(b + 1) * T][:, sl]
                nc.vector.tensor_tensor(ob, relu_f[:, sl],
                                        zslot(1, b, h * H, H), ALU.add)
                nc.sync.dma_start(out_d[b][:, sl], ob)

        # emit batch 0's tail right after its last chunk so its psum
        # slots rotate at the right position and the whole chain (which has
        # no PE dependence until Wo) hides under batch 1's softmax.
        for j, b in _steps:
            emit_att_step(j, b)
            if (j, b) == (1, 0):
                emit_qkv(1)
            if (j, b) == (7, 0):
                emit_attn_tail(0)
            if (j, b) == (3, 1):
                emit_ln_tail(5, 0, stats_a[0])
        psf0 = emit_fc_mm(0)
        emit_attn_tail(1)
        emit_fc_evict(0, psf0)
        psf1 = emit_fc_mm(1)
        emit_ln_tail(5, 1, stats_a[1])
        emit_fc_evict(1, psf1)

    nc.compile()
    return nc


def _build(flags: _Flags):
    nc = _Bacc("TRN2", target_bir_lowering=False, debug=False,
               num_devices=NCORES)

    def dram(name, shape, kind="ExternalInput", dtype=F32):
        return nc.dram_tensor(name, shape, dtype, kind=kind).ap()

    x_d = dram("x", [BPC, C, T])
    pos_d = dram("pos", [C, T], dtype=BF16)
    w2_d = dram("w2", [C, NCONV * KW * C], dtype=BF16)
    wqkv_d = dram("wqkv", [C, 3 * DK], dtype=BF16)
    wo_d = dram("wo", [DK, C], dtype=BF16)
    fcw_d = dram("fcw", [C, C], dtype=BF16)
    out_d = dram("out", [BPC, C, T], kind="ExternalOutput")
    if not flags.b2_zero:
        b2_d = dram("b2", [C, NCONV])
    if not flags.fcb_zero:
        fcb_d = dram("fcb", [C, 1])
    gb_entries = ([(l, "g") for l in range(NLN) if flags.ln_gc[l] is None]
                  + [(l, "b") for l in range(NLN) if flags.ln_bc[l] is None])
    if gb_entries:
        gb_d = dram("gb", [C, len(gb_entries) * T])
    if not flags.mask_ones:
        mb_d = dram("mb", [C, BPC * (T // C)])   # [128, 2*8] key-mask exp bias
        qm_d = dram("qm", [BPC, T])              # query-mask rows

    from contextlib import ExitStack

    with tile.TileContext(nc) as tc, ExitStack() as ctx:
        cst = ctx.enter_context(tc.tile_pool(name="cst", bufs=1))
        big = ctx.enter_context(tc.tile_pool(name="bigbuf", bufs=1))
        resp = ctx.enter_context(tc.tile_pool(name="resp", bufs=2))
        work = ctx.enter_context(tc.tile_pool(name="work", bufs=2))
        tiny = ctx.enter_context(tc.tile_pool(name="tiny", bufs=4))
        # PSUM: 16KB/partition total.  ps_big: two [C,T] f32 slots (8KB);
        # ps_sm: one rotating tag sized [*,T] f32 x2 (8KB) shared by the PE
        # warm-up, LN-stats broadcasts, v-projection, PV accumulators and
        # the reciprocal broadcast.
        ps_big = ctx.enter_context(
            tc.tile_pool(name="ps_big", bufs=2, space="PSUM"))
        ps_sm = ctx.enter_context(
            tc.tile_pool(name="ps_sm", bufs=2, space="PSUM"))

        _smctr = [0]

        def sm_tile(shape):
            _smctr[0] += 1
            return ps_sm.tile(shape, F32, tag="ps_sm",
                              name=f"sm{_smctr[0]}")

        # ---- tiny constants (engine-init, no DMA dependency) ----
        ones128 = cst.tile([C, C], F32R, tag="ones128")
        nc.vector.memset(ones128[:], 1.0)
        ones_row_r = cst.tile([1, C], F32R, tag="ones_row_r")
        nc.vector.memset(ones_row_r[:], 1.0)
        ones_row_b = cst.tile([1, C], BF16, tag="ones_row_b")
        nc.vector.memset(ones_row_b[:], 1.0)
        const_tiles: dict = {}

        def const_ap(val: float, npart: int = C):
            if val == 0.0:
                return 0.0
            if val not in const_tiles:
                t = cst.tile([C, 1], F32, tag=f"cst{len(const_tiles)}")
                nc.vector.memset(t[:], val)
                const_tiles[val] = t
            return const_tiles[val][0:npart, :]

        # ---- PE warm-up: dummy matmuls during the prologue DMA keep the
        # tensor engine's p-state ramp running so real matmuls start at full
        # clock (~30 x 128-row matmuls ~= 3us from the cold clock).
        for _ in range(N_WARMUP):
            wps = sm_tile([1, C])
            nc.tensor.matmul(wps[:], ones128[:, 0:1], ones128[:],
                             start=True, stop=True)

        # ---- weights / inputs DMA (the DMA engines serialize transfers:
        # order by criticality: pos+x first (conv0 input), then layer-0 conv
        # weights, then the rest) ----
        pos_sb = cst.tile([C, T], BF16, tag="pos")
        nc.sync.dma_start(pos_sb[:], pos_d[:])
        xin = big.tile([C, BPC * T], F32, tag="xin")
        XSPL = 520            # matches the x+pos chunk boundary
        nc.sync.dma_start(xin[:, 0:XSPL], x_d[0][:, 0:XSPL])
        w2_sb = cst.tile([C, NCONV * KW * C], BF16, tag="w2")
        LW = KW * C
        nc.sync.dma_start(w2_sb[:, 0:LW], w2_d[:, 0:LW])
        nc.sync.dma_start(xin[:, XSPL:T], x_d[0][:, XSPL:T])
        for h in range(2):
            nc.sync.dma_start(xin[:, T + h * H:T + (h + 1) * H],
                              x_d[1][:, h * H:(h + 1) * H])
        for li in range(1, NCONV):
            nc.sync.dma_start(w2_sb[:, li * LW:(li + 1) * LW],
                              w2_d[:, li * LW:(li + 1) * LW])
        wqkv_sb = cst.tile([C, 3 * DK], BF16, tag="wqkv")
        nc.sync.dma_start(wqkv_sb[:], wqkv_d)
        wo_sb = cst.tile([DK, C], BF16, tag="wo")
        nc.sync.dma_start(wo_sb[:], wo_d)
        fcw_sb = cst.tile([C, C], BF16, tag="fcw")
        nc.sync.dma_start(fcw_sb[:], fcw_d)
        if not flags.b2_zero:
            b2_sb = cst.tile([C, NCONV], F32, tag="b2")
            nc.sync.dma_start(b2_sb[:], b2_d[:])
        if not flags.fcb_zero:
            fcb_sb = cst.tile([C, 1], F32, tag="fcb")
            nc.sync.dma_start(fcb_sb[:], fcb_d[:])
        if gb_entries:
            gb_sb = cst.tile([C, len(gb_entries) * T], F32, tag="gb")
            nc.sync.dma_start(gb_sb[:], gb_d[:])
            gb_ix = {e: i for i, e in enumerate(gb_entries)}

            def gb_ap(l, kind):
                i0 = gb_ix[(l, kind)] * T
                return gb_sb[:, i0:i0 + T]
        if not flags.mask_ones:
            mb_sb = cst.tile([C, BPC * (T // C)], F32, tag="mb")
            nc.sync.dma_start(mb_sb[:], mb_d[:])
            qm_sb = cst.tile([1, BPC * T], F32, tag="qm")
            for b in range(BPC):
                nc.sync.dma_start(qm_sb[:, b * T:(b + 1) * T], qm_d[b:b + 1, :])

        # conv/attention input activations, zero-padded per batch elem:
        # [0^3 | x_b (1024) | 0^3].
        xpad = big.tile([C, BPC * PADT], BF16, tag="xpad")
        for b in range(BPC):
            nc.gpsimd.memset(xpad[:, b * PADT:b * PADT + 3], 0.0)
            nc.gpsimd.memset(xpad[:, b * PADT + 3 + T:(b + 1) * PADT], 0.0)

        sq_scr = big.tile([C, T], BF16, tag="sq_scr")
        out_sb = big.tile([C, BPC * T], F32, tag="out_sb")

        def ip(b, off=0, n=T):
            """AP of the xpad interior for batch b (bf16)."""
            return xpad[:, b * PADT + 3 + off: b * PADT + 3 + off + n]

        def emit_ln_tail(ln_idx, b, stats_b, src, dst):
            """Cross-partition LN stats reduce + rs compute + normalize.

            stats_b: [C,2] f32 (col0 = per-partition sums, col1 = sums of
            squares).  src: bf16 [C,T] SBUF; dst: bf16 [C,T]."""
            psb = sm_tile([C, 2])
            nc.tensor.matmul(psb[:], ones128[:], stats_b[:].bitcast(F32R),
                             start=True, stop=True)
            mom = tiny.tile([C, 2], F32, tag="mom")     # [mu, m2]
            nc.vector.tensor_scalar_mul(mom[:], psb[:], 1.0 / NEL)
            mu = mom[:, 0:1]
            # nvar = mu*mu - m2 = -(var); folded back by Ln(scale=-1)
            nvar = tiny.tile([C, 1], F32, tag="nvar")
            nc.vector.scalar_tensor_tensor(nvar[:], mu, mu, mom[:, 1:2],
                                           op0=ALU.mult, op1=ALU.subtract)
            gc, bc = flags.ln_gc[ln_idx], flags.ln_bc[ln_idx]
            lnv = tiny.tile([C, 1], F32, tag="lnv")
            nc.scalar.activation(lnv[:], nvar[:], AF.Ln, scale=-1.0,
                                 bias=const_ap(EPS))
            rs = tiny.tile([C, 1], F32, tag="rs")
            expb = math.log(gc) if (gc is not None and gc > 0.0) else 0.0
            nc.scalar.activation(rs[:], lnv[:], AF.Exp, scale=-0.5,
                                 bias=const_ap(expb))
            if gc is not None and gc <= 0.0 and gc != 1.0:
                rs2 = tiny.tile([C, 1], F32, tag="rs2")
                nc.vector.tensor_scalar_mul(rs2[:], rs[:], gc)
                rs = rs2
            post = []
            if gc is None:
                post.append(lambda i, o: nc.vector.tensor_tensor(
                    o, i, gb_ap(ln_idx, "g"), ALU.mult))
            if bc is None:
                post.append(lambda i, o: nc.vector.tensor_tensor(
                    o, i, gb_ap(ln_idx, "b"), ALU.add))
            elif bc != 0.0:
                post.append(lambda i, o: nc.vector.tensor_scalar_add(
                    o, i, bc))
            if not post:
                # TensorScalarPtr, all-bf16 SBUF -> DVE 4x mode
                nc.vector.tensor_scalar(dst, src, mu, rs[:],
                                        op0=ALU.subtract, op1=ALU.mult)
            else:
                mids = [sq_scr[:], out_sb[:, b * T:(b + 1) * T]]
                nc.vector.tensor_scalar(mids[0], src, mu, rs[:],
                                        op0=ALU.subtract, op1=ALU.mult)
                for i, emit in enumerate(post):
                    last = (i == len(post) - 1)
                    emit(mids[i % 2], dst if last else mids[(i + 1) % 2])

        # ---- x + pos -> xpad (conv0 input) ----
        res = resp.tile([C, BPC * T], BF16, tag="res")
        stats_x = []
        for b in range(BPC):
            stats_b = tiny.tile([C, 2], F32, tag="stats")
            nc.vector.scalar_tensor_tensor(
                ip(b), xin[:, b * T:(b + 1) * T], 1.0, pos_sb[:],
                op0=ALU.mult, op1=ALU.add, accum_out=stats_b[:, 0:1])
            nc.scalar.activation(sq_scr[:], ip(b), AF.Square,
                                 accum_out=stats_b[:, 1:2])
            stats_x.append(stats_b)

        # ---- conv layers.  Software-pipelined emission so the PE queue
        # never stalls behind LN-stats matmuls:
        #   mm_b0(l) | tail(l-1,b1) | evict_b0(l) | mm_b1(l) | tail(l,b0)
        #   | evict_b1(l) -> pending tail(l,b1)
        # (LN0's tails slot in after conv0's batch-0 matmuls.) ----
        def emit_conv_mm(li, b):
            psc = ps_big.tile([C, T], F32, tag="ps_main")
            for h in range(2):
                for d in range(KW):
                    nc.tensor.matmul(
                        psc[:, h * H:(h + 1) * H],
                        w2_sb[:, (li * KW + d) * C:(li * KW + d + 1) * C],
                        xpad[:, b * PADT + h * H + d:
                             b * PADT + h * H + d + H],
                        start=(d == 0), stop=(d == KW - 1))
            return psc

        def emit_conv_evict(li, b, psc, new_res, res):
            stats_b = tiny.tile([C, 2], F32, tag="stats")
            nr = new_res[:, b * T:(b + 1) * T]
            if flags.b2_zero:
                nc.vector.scalar_tensor_tensor(
                    nr, psc[:], 0.0, res[:, b * T:(b + 1) * T],
                    op0=ALU.max, op1=ALU.add, accum_out=stats_b[:, 0:1])
            else:
                relu_t = work.tile([C, T], BF16, tag="relu_t")
                nc.scalar.activation(relu_t[:], psc[:], AF.Relu,
                                     bias=b2_sb[:, li:li + 1])
                nc.vector.scalar_tensor_tensor(
                    nr, relu_t[:], 1.0, res[:, b * T:(b + 1) * T],
                    op0=ALU.mult, op1=ALU.add, accum_out=stats_b[:, 0:1])
            nc.scalar.activation(sq_scr[:], nr, AF.Square,
                                 accum_out=stats_b[:, 1:2])
            return stats_b

        pending = None     # (ln_idx, b, stats, src) for batch 1's LN tail
        for li in range(NCONV):
            new_res = resp.tile([C, BPC * T], BF16, tag="res")
            psc0 = emit_conv_mm(li, 0)
            if li == 0:
                for b in range(BPC):
                    emit_ln_tail(0, b, stats_x[b], ip(b),
                                 res[:, b * T:(b + 1) * T])
            elif pending is not None:
                emit_ln_tail(*pending)
                pending = None
            st0 = emit_conv_evict(li, 0, psc0, new_res, res)
            psc1 = emit_conv_mm(li, 1)
            emit_ln_tail(1 + li, 0, st0, new_res[:, 0:T], ip(0))
            st1 = emit_conv_evict(li, 1, psc1, new_res, res)
            pending = (1 + li, 1, st1, new_res[:, T:2 * T], ip(1))
            res = new_res

        # ---- attention: both batch elems chunk-interleaved ----
        new_res = resp.tile([C, BPC * T], BF16, tag="res")
        qk = [None] * BPC
        vt = [None] * BPC
        eT = [None] * BPC
        psa = [None] * BPC
        stats_a = [None] * BPC

        def emit_qkv(b):
            xa = ip(b)
            # q and k projections ([DK,T] psums on partitions 0:64; the PE
            # requires equal base partitions for stationary and moving).
            psq = ps_big.tile([DK, T], F32, tag="ps_main")
            for h in range(2):
                nc.tensor.matmul(psq[:, h * H:(h + 1) * H],
                                 wqkv_sb[:, 0:DK],
                                 xa[:, h * H:h * H + H],
                                 start=True, stop=True)
            psk = ps_big.tile([DK, T], F32, tag="ps_main")
            for h in range(2):
                nc.tensor.matmul(psk[:, h * H:(h + 1) * H],
                                 wqkv_sb[:, DK:2 * DK],
                                 xa[:, h * H:h * H + H],
                                 start=True, stop=True)
            qk_b = work.tile([DK, 2 * T], BF16, tag="qk")
            nc.vector.tensor_copy(qk_b[:, 0:T], psq[:])
            nc.scalar.copy(qk_b[:, T:2 * T], psk[:])
            qk[b] = qk_b
            # v in [t, d] layout
            psv = sm_tile([C, H])
            for j in range(8):
                nc.tensor.matmul(psv[:, j * DK:(j + 1) * DK],
                                 xa[:, j * C:(j + 1) * C],
                                 wqkv_sb[:, 2 * DK:3 * DK],
                                 start=True, stop=True)
            vt_b = work.tile([C, 8, DK + 1], BF16, tag="vt")
            nc.gpsimd.memset(vt_b[:, :, DK:DK + 1], 1.0)
            nc.vector.tensor_copy(
                vt_b[:, :, 0:DK],
                psv[:].rearrange("p (j k) -> p j k", k=DK))
            vt[b] = vt_b
            eT[b] = work.tile([C, 8 * T], BF16, tag="eT", name=f"eT{b}")
            stats_a[b] = tiny.tile([C, 2], F32, tag="stats", name=f"sta{b}")

        emit_qkv(0)
        if pending is not None:          # batch 1's LN after conv3
            emit_ln_tail(*pending)
            pending = None
        emit_qkv(1)

        # scores + exp + PV, j-chunk-interleaved across both batches
        for j in range(8):
            for b in range(BPC):
                pss = ps_big.tile([C, T], F32, tag="ps_main")
                for h in range(2):
                    nc.tensor.matmul(pss[:, h * H:(h + 1) * H],
                                     qk[b][:, T + j * C:T + (j + 1) * C],
                                     qk[b][:, h * H:h * H + H],
                                     start=True, stop=True)
                if flags.mask_ones:
                    nc.scalar.activation(eT[b][:, j * T:(j + 1) * T], pss[:],
                                         AF.Exp)
                else:
                    nc.scalar.activation(eT[b][:, j * T:(j + 1) * T], pss[:],
                                         AF.Exp,
                                         bias=mb_sb[:, b * 8 + j:b * 8 + j + 1])
                if j == 0:
                    psa[b] = ps_sm.tile([DK + 1, T], F32, tag="ps_sm",
                                        name=f"psa{b}")
                for h in range(2):
                    nc.tensor.matmul(
                        psa[b][:, h * H:(h + 1) * H], vt[b][:, j, :],
                        eT[b][:, j * T + h * H: j * T + h * H + H],
                        start=(j == 0), stop=(j == 7))

        # attention tails: first both batches' psa reads (so the rotating
        # ps_sm slots holding psa are not reclaimed while still unread),
        # then broadcast/normalize/Wo/residual per batch, then LNe + FC.
        avs = [None] * BPC
        rrs = [None] * BPC

        def emit_attn_psa_read(b):
            avs[b] = work.tile([DK, T], BF16, tag="av", name=f"av{b}")
            rrs[b] = tiny.tile([1, T], F32, tag="rr", bufs=2, name=f"rr{b}")
            if flags.mask_ones:
                nc.vector.reciprocal(rrs[b][:], psa[b][DK:DK + 1, :])
            else:
                # guard fully-masked queries (denom==0): denom+1e-30
                dn = tiny.tile([1, T], F32, tag="dn", bufs=2)
                nc.vector.tensor_scalar_add(dn[:], psa[b][DK:DK + 1, :],
                                            1e-30)
                nc.vector.reciprocal(rrs[b][:], dn[:])
            nc.vector.tensor_copy(avs[b][:], psa[b][0:DK, :])

        def emit_attn_tail(b):
            av, rr = avs[b], rrs[b]
            psr = sm_tile([DK, T])
            avn = work.tile([DK, T], BF16, tag="avn")
            pso = ps_big.tile([C, T], F32, tag="ps_main")
            for h in range(2):
                sl = slice(h * H, (h + 1) * H)
                nc.tensor.matmul(psr[:, sl], ones_row_r[:, 0:DK],
                                 rr[:, sl].bitcast(F32R), start=True,
                                 stop=True)
                nc.vector.tensor_tensor(avn[:, sl], av[:, sl],
                                        psr[:, sl], ALU.mult)
                nc.tensor.matmul(pso[:, sl], wo_sb[:], avn[:, sl],
                                 start=True, stop=True)
            nr = new_res[:, b * T:(b + 1) * T]
            if flags.mask_ones:
                nc.vector.scalar_tensor_tensor(
                    nr, pso[:], 1.0, res[:, b * T:(b + 1) * T],
                    op0=ALU.mult, op1=ALU.add, accum_out=stats_a[b][:, 0:1])
            else:
                qmb = work.tile([C, T], F32, tag="qmb")
                att = work.tile([C, T], F32, tag="att_m")
                for h in range(2):
                    psm = sm_tile([C, H])
                    nc.tensor.matmul(psm[:], ones_row_r[:],
                                     qm_sb[:, b * T + h * H:
                                           b * T + (h + 1) * H].bitcast(F32R),
                                     start=True, stop=True)
                    nc.scalar.copy(qmb[:, h * H:(h + 1) * H], psm[:])
                nc.vector.tensor_tensor(att[:], pso[:], qmb[:], ALU.mult)
                nc.vector.scalar_tensor_tensor(
                    nr, att[:], 1.0, res[:, b * T:(b + 1) * T],
                    op0=ALU.mult, op1=ALU.add, accum_out=stats_a[b][:, 0:1])
            nc.scalar.activation(sq_scr[:], nr, AF.Square,
                                 accum_out=stats_a[b][:, 1:2])

        def emit_fc(b):
            psf = ps_big.tile([C, T], F32, tag="ps_main")
            for h in range(2):
                nc.tensor.matmul(psf[:, h * H:(h + 1) * H], fcw_sb[:],
                                 ip(b, h * H, H), start=True, stop=True)
            for h in range(2):
                sl = slice(h * H, (h + 1) * H)
                ob = out_sb[:, b * T + h * H:b * T + (h + 1) * H]
                rb_ = new_res[:, b * T + h * H:b * T + (h + 1) * H]
                if flags.fcb_zero:
                    nc.vector.scalar_tensor_tensor(
                        ob, psf[:, sl], 0.0, rb_, op0=ALU.max, op1=ALU.add)
                else:
                    relu_t = work.tile([C, T], F32, tag="relu_f")
                    nc.scalar.activation(relu_t[:, sl], psf[:, sl], AF.Relu,
                                         bias=fcb_sb[:])
                    nc.vector.tensor_tensor(ob, relu_t[:, sl], rb_, ALU.add)
                nc.sync.dma_start(out_d[b][:, sl], ob)

        emit_attn_psa_read(0)
        emit_attn_psa_read(1)
        emit_attn_tail(0)
        emit_attn_tail(1)
        emit_ln_tail(NLN - 1, 0, stats_a[0], new_res[:, 0:T], ip(0))
        emit_fc(0)
        emit_ln_tail(NLN - 1, 1, stats_a[1], new_res[:, T:2 * T], ip(1))
        emit_fc(1)

    nc.compile()
    return nc


_CACHE: dict = {}
LAST_RUN: dict = {}   # exposed for test harnesses (nc, in_maps)


def kernel(x, mask, dw_w, dw_b, pw_w, pw_b, norm0_g, norm0_b,
           norms_g, norms_b, norme_g, norme_b,
           Wq, Wk, Wv, Wo, fc_w, fc_b):
    import ml_dtypes
    BF = ml_dtypes.bfloat16
    x = np.asarray(x, dtype=np.float32)
    mask = np.asarray(mask, dtype=np.float32)

    # ---- host-side constant folding ----
    w2 = np.empty((C, NCONV, KW, C), dtype=np.float32)
    for i in range(NCONV):
        pwT = np.asarray(pw_w[i], np.float32).T          # [c, o]
        for d in range(KW):
            w2[:, i, d, :] = pwT * np.asarray(dw_w[i][:, d],
                                              np.float32)[:, None]
    w2 = w2.reshape(C, NCONV * KW * C)
    b2 = np.stack([np.asarray(pw_w[i], np.float32)
                   @ np.asarray(dw_b[i], np.float32)
                   + np.asarray(pw_b[i], np.float32)
                   for i in range(NCONV)], axis=1)        # [C, NCONV]
    wqkv = np.concatenate([np.asarray(Wq, np.float32) / math.sqrt(DK),
                           np.asarray(Wk, np.float32),
                           np.asarray(Wv, np.float32)], axis=1)  # [C, 3*DK]
    wo = np.asarray(Wo, np.float32)
    wo_eff = np.ascontiguousarray(wo[:DK] + wo[DK:])      # [DK, C]
    fcw = np.ascontiguousarray(np.asarray(fc_w, np.float32).T)  # [c, o]
    fcb = np.asarray(fc_b, np.float32).reshape(C, 1)
    pos = _pos_encoding()

    gs = [norm0_g] + [norms_g[i] for i in range(NCONV)] + [norme_g]
    bs = [norm0_b] + [norms_b[i] for i in range(NCONV)] + [norme_b]
    ln_gc = [_uniform_val(np.asarray(g, np.float32)) for g in gs]
    ln_bc = [_uniform_val(np.asarray(bb, np.float32)) for bb in bs]
    flags = _Flags(ln_gc, ln_bc,
                   b2_zero=not b2.any(),
                   fcb_zero=not fcb.any(),
                   mask_ones=bool(np.all(mask == 1.0)))

    fast = (flags.mask_ones and flags.b2_zero and flags.fcb_zero
            and all(v is not None for v in ln_gc)
            and all(v is not None for v in ln_bc))

    key = (fast,) + flags.key()
    if key not in _CACHE:
        _CACHE[key] = _build_fast(flags) if fast else _build(flags)
    nc = _CACHE[key]

    base = {"pos": pos.astype(BF), "w2": w2.astype(BF),
            "wqkv": wqkv.astype(BF), "wo": wo_eff.astype(BF),
            "fcw": fcw.astype(BF)}
    if fast:
        w2r = w2.reshape(C, NCONV, KW, C)
        sums = np.zeros((C, 7), np.float32)
        for li in (1, 2, 3):
            sums[:, li - 1] = w2r[:, li].sum(axis=(0, 1))   # rowsum per o
        sums[:DK, 3] = wqkv[:, 0:DK].sum(axis=0)            # wqsum
        sums[:DK, 4] = wqkv[:, DK:2 * DK].sum(axis=0)       # wksum
        wvsum = wqkv[:, 2 * DK:3 * DK].sum(axis=0)          # [DK]
        sums[:, 5] = wo_eff.T @ wvsum                       # wowvsum
        sums[:, 6] = fcw.sum(axis=0)                        # fcsum
        base["sums"] = np.ascontiguousarray(sums)
    if not flags.b2_zero:
        base["b2"] = np.ascontiguousarray(b2)
    if not flags.fcb_zero:
        base["fcb"] = np.ascontiguousarray(fcb)
    gb_entries = ([(l, "g") for l in range(NLN) if flags.ln_gc[l] is None]
                  + [(l, "b") for l in range(NLN) if flags.ln_bc[l] is None])
    if gb_entries:
        gb = np.empty((C, len(gb_entries) * T), np.float32)
        for i, (l, kind) in enumerate(gb_entries):
            src = gs[l] if kind == "g" else bs[l]
            gb[:, i * T:(i + 1) * T] = np.asarray(src, np.float32)
        base["gb"] = gb

    in_maps = []
    for c in range(NCORES):
        m = dict(base)
        m["x"] = np.ascontiguousarray(x[c * BPC:(c + 1) * BPC])
        if not flags.mask_ones:
            msk = mask[c * BPC:(c + 1) * BPC]             # [BPC, T]
            mb = np.where(msk == 0.0, np.float32(-1e9), np.float32(0.0))
            m["mb"] = np.ascontiguousarray(
                mb.reshape(BPC, 8, C).transpose(2, 0, 1).reshape(C, BPC * 8))
            m["qm"] = np.ascontiguousarray(msk)
        in_maps.append(m)

    LAST_RUN["nc"] = nc
    LAST_RUN["in_maps"] = in_maps

    res = run_bass_kernel_spmd(nc, in_maps, list(range(NCORES)))
    out = np.concatenate([r["out"] for r in res.results], axis=0)
    return out.astype(np.float32)
